# revision 2
# baseline (speedup 1.0000x reference)
"""CAM (channel attention) module kernel for Trainium2, 8-core data-parallel.

Reference computation (per batch b, channel c):
    v = x[b,c]                         # (P=3, HW=4096)
    energy = v @ v.T                   # (3,3) Gram matrix
    en = rowmax(energy) - energy
    att = softmax(en, axis=-1)
    out = att @ v                      # (3, 4096)
    y[b,c] = gamma * out + x[b,c]

Sharding: batch dim (B=8) across the 8 NeuronCores; no cross-core comms.

Strategy (v2, bf16 + TensorE mix):
  - Host casts x to bf16 (tolerance is 2e-2; bf16 rounding costs ~1e-3) and
    the kernel stores y as bf16 -> 12.6 MB HBM traffic/core, ~35 us roofline.
  - Per core: 2 groups of 128 channels on SBUF partitions, free dim = P*HW.
  - Gram 3x3: cross terms = DVE tensor_tensor mult (2x_1p bf16) + tensor_scalar
    accum (4x bf16); diagonals split between ACT Square+accum and the DVE pair
    (scalar_tensor_tensor is 1x-only on this HW -- avoided entirely).
  - softmax: tiny [128, 3x3] ops, rowmin trick (shift-invariant, exponents <=0).
  - mix (att @ v + residual): folded coeffs Cf = gamma*att + I applied on the
    OTHERWISE-IDLE TensorE: per (row i, path j) a diagonal stationary
    W_ij = diag(Cf[:, i, j]) (built in ~94 ns as identity x per-partition
    scalar on DVE), then PSUM accumulates y_i = sum_j W_ij @ v_j over 3
    matmuls per 512-col chunk.  PSUM (f32) is drained to bf16 SBUF tiles by
    ACT/DVE copies (split tunable) and DMA'd out.
"""

import numpy as np
import ml_dtypes

import concourse.bacc as bacc
import concourse.mybir as mybir
import concourse.tile as tile
from concourse.bass_utils import run_bass_kernel_spmd

B, C, P, H, W = 8, 256, 3, 64, 64
HW = H * W
N_CORES = 8
PARTS = 128
NCHUNK = 512            # matmul moving free size (one PSUM bank of f32 out)
HALF = 2048             # drain granularity (4 PSUM banks)

F32 = mybir.dt.float32
BF16 = mybir.dt.bfloat16
Alu = mybir.AluOpType
Act = mybir.ActivationFunctionType

BF16_NP = ml_dtypes.bfloat16


def build_nc(C_=C, HW_=HW, repeat=1, n_diag_act=3, n_drain_act=6):
    """Build the per-core Bass program.

    Each core sees x:(C_,P,HW_) bf16, gamma:(1,1) f32, ident:(128,128) bf16.
    Produces y:(C_,P,HW_) bf16.

    repeat>1 re-runs the whole computation (same I/O, idempotent) that many
    times in one program -- used by test.py to time the kernel by slope.
    n_diag_act: how many of the 3 per-group Gram diagonals go to ScalarE
    (rest use the DVE tt+ts pair).
    n_drain_act: how many of the 6 per-group PSUM half-row drains go to
    ScalarE (rest on DVE).
    """
    assert C_ % PARTS == 0
    n_groups = C_ // PARTS
    n_half = HW_ // HALF
    n_chunk_per_half = HALF // NCHUNK

    nc = bacc.Bacc("TRN2", target_bir_lowering=False, debug=False)

    x_d = nc.dram_tensor("x", [C_, P, HW_], BF16, kind="ExternalInput")
    g_d = nc.dram_tensor("gamma", [1, 1], F32, kind="ExternalInput")
    id_d = nc.dram_tensor("ident", [PARTS, PARTS], BF16, kind="ExternalInput")
    y_d = nc.dram_tensor("y", [C_, P, HW_], BF16, kind="ExternalOutput")

    with tile.TileContext(nc) as tc:
        with (
            tc.tile_pool(name="consts", bufs=1) as consts,
            tc.tile_pool(name="vpool", bufs=2) as vpool,
            tc.tile_pool(name="scratch", bufs=1) as scratch,
            tc.tile_pool(name="wpool", bufs=2) as wpool,
            tc.tile_pool(name="tpool", bufs=4) as tpool,
            tc.tile_pool(name="smalls", bufs=2) as smalls,
            tc.psum_pool(name="pspool", bufs=2) as pspool,
        ):
            # --- constants (once) ---
            gsb = consts.tile([1, 1], F32)
            nc.sync.dma_start(gsb[:], g_d[:])
            gamma_bc = consts.tile([PARTS, 1], F32)
            nc.gpsimd.partition_broadcast(gamma_bc[:], gsb[:])

            identPE = consts.tile([PARTS, PARTS], BF16)
            nc.sync.dma_start(identPE[:], id_d[:])

            ident9 = consts.tile([PARTS, 9], F32)
            nc.vector.memset(ident9[:], 0.0)
            for i in range(P):
                nc.vector.memset(ident9[:, 4 * i : 4 * i + 1], 1.0)

            for g in range(n_groups * repeat):
                g = g % n_groups
                cs = slice(g * PARTS, (g + 1) * PARTS)

                # --- load group: per-path DMAs so compute starts early ---
                v = vpool.tile([PARTS, P, HW_], BF16)
                for i in range(P):
                    nc.sync.dma_start(v[:, i, :], x_d[cs, i, :])

                # --- phase 1: per-channel 3x3 Gram matrix over HW ---
                E = smalls.tile([PARTS, 9], F32)
                # diagonals: ACT Square+accum (1x) or DVE tt*+ts-accum pair
                for i in range(P):
                    if i < n_diag_act:
                        scr = scratch.tile([PARTS, HW_], BF16, tag="scr_act", bufs=1)
                        nc.scalar.activation(
                            scr[:], v[:, i, :], Act.Square,
                            accum_out=E[:, 4 * i : 4 * i + 1],
                        )
                    else:
                        prod = scratch.tile([PARTS, HW_], BF16, tag="scr_dve", bufs=2)
                        nc.vector.tensor_tensor(prod[:], v[:, i, :], v[:, i, :], op=Alu.mult)
                        nc.vector.tensor_scalar(
                            prod[:], prod[:], 1.0, None, op0=Alu.mult,
                            accum_out=E[:, 4 * i : 4 * i + 1],
                        )
                # cross terms on DVE: tt mult (2x_1p bf16) + ts accum (4x bf16)
                for i, j, col in ((0, 1, 1), (1, 2, 5), (0, 2, 2)):
                    prod = scratch.tile([PARTS, HW_], BF16, tag="scr_dve", bufs=2)
                    nc.vector.tensor_tensor(prod[:], v[:, i, :], v[:, j, :], op=Alu.mult)
                    nc.vector.tensor_scalar(
                        prod[:], prod[:], 1.0, None, op0=Alu.mult,
                        accum_out=E[:, col : col + 1],
                    )
                # mirror symmetric entries: (1,0)<-(0,1), (2,1)<-(1,2), (2,0)<-(0,2)
                for src, dst in ((1, 3), (5, 7), (2, 6)):
                    nc.scalar.copy(E[:, dst : dst + 1], E[:, src : src + 1])

                # --- softmax over rows of the 3x3, coeffs Cf = gamma*att + I ---
                E3 = E.rearrange("p (i j) -> p i j", j=P)
                M = smalls.tile([PARTS, P, 1], F32)
                # reference computes softmax(rowmax - E); softmax is shift
                # invariant, so use (rowmin - E): exponents stay <= 0.
                nc.vector.tensor_reduce(M[:], E3, axis=mybir.AxisListType.X, op=Alu.min)
                EX = smalls.tile([PARTS, P, P], F32)
                for i in range(P):
                    nc.scalar.activation(
                        EX[:, i, :], E3[:, i, :], Act.Exp,
                        scale=-1.0, bias=M[:, i, 0:1],
                    )
                S = smalls.tile([PARTS, P, 1], F32)
                nc.vector.tensor_reduce(S[:], EX[:], axis=mybir.AxisListType.X, op=Alu.add)
                R = smalls.tile([PARTS, P, 1], F32)
                nc.vector.reciprocal(R[:], S[:])
                A = smalls.tile([PARTS, P, P], F32)
                nc.vector.tensor_mul(A[:], EX[:], R[:].broadcast_to([PARTS, P, P]))
                Cf = smalls.tile([PARTS, 9], F32)
                nc.vector.scalar_tensor_tensor(
                    Cf[:].rearrange("p (i j) -> p i j", j=P), A[:], gamma_bc[:, 0:1],
                    ident9[:].rearrange("p (i j) -> p i j", j=P),
                    op0=Alu.mult, op1=Alu.add,
                )

                # --- stationaries: W[k] = diag(Cf[:, k]) = ident * Cf_col ---
                Wst = wpool.tile([PARTS, 9, PARTS], BF16)
                for k in range(9):
                    nc.vector.tensor_scalar(
                        Wst[:, k, :], identPE[:], Cf[:, k : k + 1], None, op0=Alu.mult,
                    )

                # --- phase 2 on TensorE: y_i = sum_j diag(Cf[i,j]) @ v_j ---
                drain_idx = 0
                for i in range(P):
                    for h in range(n_half):
                        ps = pspool.tile([PARTS, HALF], F32)
                        for j in range(P):
                            for cch in range(n_chunk_per_half):
                                lo = h * HALF + cch * NCHUNK
                                nc.tensor.matmul(
                                    ps[:, cch * NCHUNK : (cch + 1) * NCHUNK],
                                    Wst[:, 3 * i + j, :],
                                    v[:, j, lo : lo + NCHUNK],
                                    start=(j == 0),
                                    stop=(j == P - 1),
                                )
                        t = tpool.tile([PARTS, HALF], BF16, tag="t", bufs=4)
                        if drain_idx < n_drain_act:
                            nc.scalar.copy(t[:], ps[:])
                        else:
                            nc.vector.tensor_copy(t[:], ps[:])
                        drain_idx += 1
                        nc.sync.dma_start(
                            y_d[cs, i, h * HALF : (h + 1) * HALF], t[:]
                        )

    nc.compile()
    return nc


_NC_CACHE = {}


def _get_nc(C_=C, HW_=HW):
    key = (C_, HW_)
    if key not in _NC_CACHE:
        _NC_CACHE[key] = build_nc(C_, HW_)
    return _NC_CACHE[key]


_IDENT = np.eye(PARTS, dtype=BF16_NP)


def make_in_maps(x: np.ndarray, gamma: np.ndarray):
    """Host-side prep: shard over batch, cast x to bf16, add ident const."""
    x_bf = np.asarray(x, dtype=BF16_NP)
    gamma = np.asarray(gamma, dtype=np.float32)
    return [
        {
            "x": np.ascontiguousarray(x_bf[k]).reshape(C, P, HW),
            "gamma": gamma.reshape(1, 1),
            "ident": _IDENT,
        }
        for k in range(N_CORES)
    ]


def run_full(x: np.ndarray, gamma: np.ndarray, **runner_kwargs):
    """Run on all 8 cores; returns the raw BassKernelResults."""
    assert np.asarray(x).shape == (B, C, P, H, W)
    nc = _get_nc()
    in_maps = make_in_maps(x, gamma)
    return run_bass_kernel_spmd(
        nc, in_maps, core_ids=list(range(N_CORES)), **runner_kwargs
    )


def kernel(x: np.ndarray, gamma: np.ndarray) -> np.ndarray:
    res = run_full(x, gamma)
    y = np.stack(
        [np.asarray(res.results[k]["y"], dtype=BF16_NP) for k in range(N_CORES)]
    )
    return y.reshape(B, C, P, H, W).astype(np.float32)


# revision 3
# speedup vs baseline: 1.5380x; 1.5380x over previous
"""CAM (channel attention) module kernel for Trainium2, 8-core data-parallel.

Reference computation (per batch b, channel c):
    v = x[b,c]                         # (P=3, HW=4096)
    energy = v @ v.T                   # (3,3) Gram matrix
    en = rowmax(energy) - energy
    att = softmax(en, axis=-1)
    out = att @ v                      # (3, 4096)
    y[b,c] = gamma * out + x[b,c]

Sharding: batch dim (B=8) across the 8 NeuronCores; no cross-core comms.

Strategy (v2, bf16 + TensorE mix):
  - Host casts x to bf16 (tolerance is 2e-2; bf16 rounding costs ~1e-3) and
    the kernel stores y as bf16 -> 12.6 MB HBM traffic/core, ~35 us roofline.
  - Per core: 2 groups of 128 channels on SBUF partitions, free dim = P*HW.
  - Gram 3x3: cross terms = DVE tensor_tensor mult (2x_1p bf16) + tensor_scalar
    accum (4x bf16); diagonals split between ACT Square+accum and the DVE pair
    (scalar_tensor_tensor is 1x-only on this HW -- avoided entirely).
  - softmax: tiny [128, 3x3] ops, rowmin trick (shift-invariant, exponents <=0).
  - mix (att @ v + residual): folded coeffs Cf = gamma*att + I applied on the
    OTHERWISE-IDLE TensorE: per (row i, path j) a diagonal stationary
    W_ij = diag(Cf[:, i, j]) (built in ~94 ns as identity x per-partition
    scalar on DVE), then PSUM accumulates y_i = sum_j W_ij @ v_j over 3
    matmuls per 512-col chunk.  PSUM (f32) is drained to bf16 SBUF tiles by
    ACT/DVE copies (split tunable) and DMA'd out.
"""

import numpy as np
import ml_dtypes

import concourse.bacc as bacc
import concourse.mybir as mybir
import concourse.tile as tile
from concourse.bass_utils import run_bass_kernel_spmd

B, C, P, H, W = 8, 256, 3, 64, 64
HW = H * W
N_CORES = 8
PARTS = 128
NCHUNK = 512            # matmul moving free size (one PSUM bank of f32 out)
HALF = 2048             # drain granularity (4 PSUM banks)

F32 = mybir.dt.float32
BF16 = mybir.dt.bfloat16
Alu = mybir.AluOpType
Act = mybir.ActivationFunctionType

BF16_NP = ml_dtypes.bfloat16


def build_nc(C_=C, HW_=HW, repeat=1, n_diag_act=3, n_drain_act=6):
    """Build the per-core Bass program.

    Each core sees x:(C_,P,HW_) bf16, gamma:(1,1) f32, ident:(128,128) bf16.
    Produces y:(C_,P,HW_) bf16.

    repeat>1 re-runs the whole computation (same I/O, idempotent) that many
    times in one program -- used by test.py to time the kernel by slope.
    n_diag_act: how many of the 3 per-group Gram diagonals go to ScalarE
    (rest use the DVE tt+ts pair).
    n_drain_act: how many of the 6 per-group PSUM half-row drains go to
    ScalarE (rest on DVE).
    """
    assert C_ % PARTS == 0
    n_groups = C_ // PARTS
    n_half = HW_ // HALF
    n_chunk_per_half = HALF // NCHUNK

    nc = bacc.Bacc("TRN2", target_bir_lowering=False, debug=False)

    x_d = nc.dram_tensor("x", [C_, P, HW_], BF16, kind="ExternalInput")
    g_d = nc.dram_tensor("gamma", [1, 1], F32, kind="ExternalInput")
    id_d = nc.dram_tensor("ident", [PARTS, PARTS], BF16, kind="ExternalInput")
    y_d = nc.dram_tensor("y", [C_, P, HW_], BF16, kind="ExternalOutput")

    with tile.TileContext(nc) as tc:
        with (
            tc.tile_pool(name="consts", bufs=1) as consts,
            tc.tile_pool(name="vpool", bufs=2) as vpool,
            tc.tile_pool(name="scratch", bufs=1) as scratch,
            tc.tile_pool(name="wpool", bufs=2) as wpool,
            tc.tile_pool(name="tpool", bufs=4) as tpool,
            tc.tile_pool(name="smalls", bufs=2) as smalls,
            tc.psum_pool(name="pspool", bufs=2) as pspool,
        ):
            # --- constants (once) ---
            gsb = consts.tile([1, 1], F32)
            nc.sync.dma_start(gsb[:], g_d[:])
            gamma_bc = consts.tile([PARTS, 1], F32)
            nc.gpsimd.partition_broadcast(gamma_bc[:], gsb[:])

            identPE = consts.tile([PARTS, PARTS], BF16)
            nc.sync.dma_start(identPE[:], id_d[:])

            ident9 = consts.tile([PARTS, 9], F32)
            nc.vector.memset(ident9[:], 0.0)
            for i in range(P):
                nc.vector.memset(ident9[:, 4 * i : 4 * i + 1], 1.0)

            for g in range(n_groups * repeat):
                g = g % n_groups
                cs = slice(g * PARTS, (g + 1) * PARTS)

                # --- load group: per-path DMAs so compute starts early ---
                v = vpool.tile([PARTS, P, HW_], BF16)
                for i in range(P):
                    nc.sync.dma_start(v[:, i, :], x_d[cs, i, :])

                # --- phase 1: per-channel 3x3 Gram matrix over HW ---
                E = smalls.tile([PARTS, 9], F32)
                # diagonals: ACT Square+accum (1x) or DVE tt*+ts-accum pair
                for i in range(P):
                    if i < n_diag_act:
                        scr = scratch.tile([PARTS, HW_], BF16, tag="scr_act", bufs=1)
                        nc.scalar.activation(
                            scr[:], v[:, i, :], Act.Square,
                            accum_out=E[:, 4 * i : 4 * i + 1],
                        )
                    else:
                        prod = scratch.tile([PARTS, HW_], BF16, tag="scr_dve", bufs=2)
                        nc.vector.tensor_tensor(prod[:], v[:, i, :], v[:, i, :], op=Alu.mult)
                        nc.vector.tensor_scalar(
                            prod[:], prod[:], 1.0, 0.0, op0=Alu.mult, op1=Alu.add,
                            accum_out=E[:, 4 * i : 4 * i + 1],
                        )
                # cross terms on DVE: tt mult (2x_1p bf16) + ts accum (4x bf16)
                for i, j, col in ((0, 1, 1), (1, 2, 5), (0, 2, 2)):
                    prod = scratch.tile([PARTS, HW_], BF16, tag="scr_dve", bufs=2)
                    nc.vector.tensor_tensor(prod[:], v[:, i, :], v[:, j, :], op=Alu.mult)
                    nc.vector.tensor_scalar(
                        prod[:], prod[:], 1.0, 0.0, op0=Alu.mult, op1=Alu.add,
                        accum_out=E[:, col : col + 1],
                    )
                # mirror symmetric entries: (1,0)<-(0,1), (2,1)<-(1,2), (2,0)<-(0,2)
                for src, dst in ((1, 3), (5, 7), (2, 6)):
                    nc.scalar.copy(E[:, dst : dst + 1], E[:, src : src + 1])

                # --- softmax over rows of the 3x3, coeffs Cf = gamma*att + I ---
                E3 = E.rearrange("p (i j) -> p i j", j=P)
                M = smalls.tile([PARTS, P, 1], F32)
                # reference computes softmax(rowmax - E); softmax is shift
                # invariant, so use (rowmin - E): exponents stay <= 0.
                nc.vector.tensor_reduce(M[:], E3, axis=mybir.AxisListType.X, op=Alu.min)
                EX = smalls.tile([PARTS, P, P], F32)
                for i in range(P):
                    nc.scalar.activation(
                        EX[:, i, :], E3[:, i, :], Act.Exp,
                        scale=-1.0, bias=M[:, i, 0:1],
                    )
                S = smalls.tile([PARTS, P, 1], F32)
                nc.vector.tensor_reduce(S[:], EX[:], axis=mybir.AxisListType.X, op=Alu.add)
                R = smalls.tile([PARTS, P, 1], F32)
                nc.vector.reciprocal(R[:], S[:])
                A = smalls.tile([PARTS, P, P], F32)
                nc.vector.tensor_mul(A[:], EX[:], R[:].broadcast_to([PARTS, P, P]))
                Cf = smalls.tile([PARTS, 9], F32)
                nc.vector.scalar_tensor_tensor(
                    Cf[:].rearrange("p (i j) -> p i j", j=P), A[:], gamma_bc[:, 0:1],
                    ident9[:].rearrange("p (i j) -> p i j", j=P),
                    op0=Alu.mult, op1=Alu.add,
                )

                # --- stationaries: W[k] = diag(Cf[:, k]) = ident * Cf_col ---
                Wst = wpool.tile([PARTS, 9, PARTS], BF16)
                for k in range(9):
                    nc.vector.tensor_scalar(
                        Wst[:, k, :], identPE[:], Cf[:, k : k + 1], None, op0=Alu.mult,
                    )

                # --- phase 2 on TensorE: y_i = sum_j diag(Cf[i,j]) @ v_j ---
                drain_idx = 0
                for i in range(P):
                    for h in range(n_half):
                        ps = pspool.tile([PARTS, HALF], F32)
                        for j in range(P):
                            for cch in range(n_chunk_per_half):
                                lo = h * HALF + cch * NCHUNK
                                nc.tensor.matmul(
                                    ps[:, cch * NCHUNK : (cch + 1) * NCHUNK],
                                    Wst[:, 3 * i + j, :],
                                    v[:, j, lo : lo + NCHUNK],
                                    start=(j == 0),
                                    stop=(j == P - 1),
                                )
                        t = tpool.tile([PARTS, HALF], BF16, tag="t", bufs=4)
                        if drain_idx < n_drain_act:
                            nc.scalar.copy(t[:], ps[:])
                        else:
                            nc.vector.tensor_copy(t[:], ps[:])
                        drain_idx += 1
                        nc.sync.dma_start(
                            y_d[cs, i, h * HALF : (h + 1) * HALF], t[:]
                        )

    nc.compile()
    return nc


_NC_CACHE = {}


def _get_nc(C_=C, HW_=HW):
    key = (C_, HW_)
    if key not in _NC_CACHE:
        _NC_CACHE[key] = build_nc(C_, HW_)
    return _NC_CACHE[key]


_IDENT = np.eye(PARTS, dtype=BF16_NP)


def make_in_maps(x: np.ndarray, gamma: np.ndarray):
    """Host-side prep: shard over batch, cast x to bf16, add ident const."""
    x_bf = np.asarray(x, dtype=BF16_NP)
    gamma = np.asarray(gamma, dtype=np.float32)
    return [
        {
            "x": np.ascontiguousarray(x_bf[k]).reshape(C, P, HW),
            "gamma": gamma.reshape(1, 1),
            "ident": _IDENT,
        }
        for k in range(N_CORES)
    ]


def run_full(x: np.ndarray, gamma: np.ndarray, **runner_kwargs):
    """Run on all 8 cores; returns the raw BassKernelResults."""
    assert np.asarray(x).shape == (B, C, P, H, W)
    nc = _get_nc()
    in_maps = make_in_maps(x, gamma)
    return run_bass_kernel_spmd(
        nc, in_maps, core_ids=list(range(N_CORES)), **runner_kwargs
    )


def kernel(x: np.ndarray, gamma: np.ndarray) -> np.ndarray:
    res = run_full(x, gamma)
    y = np.stack(
        [np.asarray(res.results[k]["y"], dtype=BF16_NP) for k in range(N_CORES)]
    )
    return y.reshape(B, C, P, H, W).astype(np.float32)
